# revision 6
# baseline (speedup 1.0000x reference)
"""MoE layer (E=8 experts, top-2 routing) on 8 Trainium2 NeuronCores.

Strategy: hidden-dim (tensor-parallel) sharding. The cheap router runs on
host in fp32 numpy, exactly reproducing the reference's softmax/top-k
semantics. The 8192 routed (token, expert) rows are sorted by expert and
padded per-expert to 16; EVERY core processes ALL rows, but only over its
own H/8 = 512 slice of each expert's hidden dim:

    core m:  h_m = gelu(x @ W1[e, 512m:512(m+1), :].T + b1 slice)
             y_m = h_m @ W2[e, :, 512m:512(m+1)].T          (partial sum)

Host sums the 8 fp16 partials and applies the top-2 gates + b2. This makes
per-core work exactly total/8 regardless of routing (no capacity padding /
load imbalance, unlike expert-parallel), at the cost of replicated
activation traffic — still far under the PE-bound compute time.

Device schedule (per core): global phase A (all experts' first matmul +
gelu into an SBUF-resident h1), then phase B (second matmul, fp16 partials
out). PE streams back-to-back with no inter-phase bubble; ACT is the only
PSUM evacuator; DVE only feeds a block of dependency-free warm-up matmuls
issued at t=0 so the PE HAM clock-gate un-throttles during the initial
DMA wait instead of during real work.

Device layout notes (per core):
  matmul computes out[m,n] = sum_p lhsT[p,m]*rhs[p,n]; contraction on the
  SBUF partition dim. All operands pre-laid-out on host so DMAs are
  contiguous per partition (8KB runs):
    xt   flat, per token-block (p=d_inner, dd, t)
    w1s  [E,P,4,8,P]   W1 slice: (d_inner, ht, dd, h_inner)
    w2s  [E,P,8,4,P]   W2 slice: (h_inner, dt, ht, d_inner)
    b1t  [P,E,4]       bias slice, partition-major
    yt   flat, per token-block (p=d_inner, dt, t)  fp16 partials
"""

import os
import sys
import types

import numpy as np

D = 1024
H = 4096
E = 8
TOPK = 2
P = 128
NCORES = 8
DT = D // P          # 8 d-tiles
HT_LOC = H // NCORES // P  # 4 local h-tiles (512 rows of H per core)
TB = 512             # psum free-dim block


def _install_axon_hooks_shim():
    """Provide antenv.axon_hooks if the container's antenv stub lacks it."""
    try:
        import antenv
    except ImportError:
        return
    if "antenv.axon_hooks" in sys.modules:
        return
    try:
        from antenv import axon_hooks  # noqa: F401
        return
    except ImportError:
        pass
    mod = types.ModuleType("antenv.axon_hooks")
    mod._hook = None

    def set_axon_ntff_profile_hook(h):
        mod._hook = h

    def get_axon_ntff_profile_hook():
        return mod._hook

    mod.set_axon_ntff_profile_hook = set_axon_ntff_profile_hook
    mod.get_axon_ntff_profile_hook = get_axon_ntff_profile_hook
    sys.modules["antenv.axon_hooks"] = mod
    antenv.axon_hooks = mod
    try:
        from trn_agent_boot.trn_boot import _ntff_profile_via_ctypes

        hook = _ntff_profile_via_ctypes("/opt/axon/libaxon_pjrt.so")
        if hook is not None:
            set_axon_ntff_profile_hook(hook)
    except Exception:
        pass


def _split_equal(c, nb):
    """Split c (a multiple of 16) into nb chunks, each a multiple of 16 —
    PE rhs reads at non-16-element-aligned SBUF offsets run measurably
    slower, so block boundaries must stay aligned."""
    base = (c // nb) // 16 * 16
    rem = c - base * nb
    extra = rem // 16
    return [base + 16 * (1 if i < extra else 0) for i in range(nb)]


def _blocks_for(c16, first_seg=False, last_seg=False):
    """Near-equal <=512 blocks (no tiny remainder blocks — those make the
    PE outrun the ACT evacuations and stall). The global last segment ends
    with a small 128 block so the kernel tail (last ACT + out DMA) is
    short."""
    c = c16
    head = []
    tail = []
    if first_seg and c16 >= 768:
        # short lead blocks: the first chain's dependency set (deps gather
        # at the accumulation-group level) is only 0.5MB, so real work
        # starts as soon as the startup DMAs land (~4.5us). The cold
        # 128-col blocks double as the HAM warm-up (no junk matmuls);
        # ramping sizes keeps the startup DMA demand under the aggregate
        # ceiling while the PE is still at 1.2GHz.
        head = [128, 128, 256]
        c -= 512
    if last_seg and c > 384:
        tail = [64]
        c -= 64
    sizes = head + (_split_equal(c, -(-c // TB)) if c > 0 else []) + tail
    out = []
    t = 0
    for tb in sizes:
        out.append((t, tb))
        t += tb
    return out


_KERNEL_CACHE = {}


def _build_kernel(c16s):
    """Build + compile the per-core Bass program for per-expert padded
    counts c16s (tuple of 8 multiples of 16)."""
    import concourse.bacc as bacc
    import concourse.mybir as mybir
    import concourse.tile as tile

    dt = mybir.dt
    seg_off = [0]
    for c in c16s:
        seg_off.append(seg_off[-1] + c)
    C = seg_off[-1]
    blocks = [
        _blocks_for(c16s[e], first_seg=(e == 0), last_seg=(e == E - 1))
        for e in range(E)
    ]

    nc = bacc.Bacc("TRN2", target_bir_lowering=False, debug=False)

    xt = nc.dram_tensor("xt", [P * C * DT], dt.float16, kind="ExternalInput")
    w1s = nc.dram_tensor("w1s", [E, P, HT_LOC, DT, P], dt.float16, kind="ExternalInput")
    w2s = nc.dram_tensor("w2s", [E, P, DT, HT_LOC, P], dt.float16, kind="ExternalInput")
    b1t = nc.dram_tensor("b1t", [P, E, HT_LOC], dt.float32, kind="ExternalInput")
    yt = nc.dram_tensor("yt", [P * C * DT], dt.float16, kind="ExternalOutput")

    with tile.TileContext(nc) as tc:
        with (
            tc.tile_pool(name="pers", bufs=1) as pers,
            tc.tile_pool(name="w1pool", bufs=2) as w1pool,
            tc.tile_pool(name="w2pool", bufs=3) as w2pool,
            tc.tile_pool(name="xtpool", bufs=4) as xtpool,
            tc.tile_pool(name="otpool", bufs=3) as otpool,
            tc.tile_pool(name="psum", bufs=8, space="PSUM") as psum,
        ):
            # --- Startup-critical DMAs, spread across BOTH HWDGE sequencers
            # (SP + ACT) and split in half so their trigger costs don't
            # serialize and the first chain's 0.5MB dep set lands ASAP. The
            # first real matmuls then run during the HAM-cold window (they
            # both do real work and warm the clock gate).
            w1_first = w1pool.tile([P, HT_LOC, DT, P], dt.float16, tag="w1_t")
            nc.sync.dma_start(w1_first[:, :1, : DT // 2, :], w1s[0][:, :1, : DT // 2, :])
            nc.scalar.dma_start(
                w1_first[:, :1, DT // 2 :, :], w1s[0][:, :1, DT // 2 :, :]
            )
            b1_sb = pers.tile([P, E, HT_LOC], dt.float32, tag="b1_sb")
            h1_all = pers.tile([P, HT_LOC, C], dt.float16, tag="h1_all")
            tb0 = blocks[0][0][1]
            xt0 = pers.tile([P, DT, tb0], dt.float16, tag="xt0")

            xt_off = {}
            off = 0
            for e in range(E):
                for (t0, tb) in blocks[e]:
                    xt_off[(e, t0)] = off
                    off += P * DT * tb

            def xt_src(e, t0, tb, d0, d1):
                base = xt_off[(e, t0)]
                src = xt[base : base + P * DT * tb].rearrange(
                    "(p d t) -> p d t", d=DT, t=tb
                )
                return src[:, d0:d1, :]

            # critical path: block 0's x split across both sequencers too
            nc.sync.dma_start(xt0[:, : DT // 2, :], xt_src(0, 0, tb0, 0, DT // 2))
            nc.scalar.dma_start(xt0[:, DT // 2 :, :], xt_src(0, 0, tb0, DT // 2, DT))
            nc.scalar.dma_start(b1_sb[:], b1t[:])

            first = True
            blkctr = 0
            for e in range(E):
                if e == 0:
                    w1_t = w1_first
                    # ht1/ht3 via SP, ht2 via ACT: the two trigger paths
                    # drain the startup burst in parallel
                    for ht in range(1, HT_LOC):
                        eng = nc.scalar if ht == 2 else nc.sync
                        eng.dma_start(
                            w1_t[:, ht : ht + 1, :, :], w1s[0][:, ht : ht + 1, :, :]
                        )
                else:
                    w1_t = w1pool.tile([P, HT_LOC, DT, P], dt.float16, tag="w1_t")
                    nc.sync.dma_start(w1_t[:], w1s[e])
                for (t0, tb) in blocks[e]:
                    blkctr += 1
                    if first:
                        xtb = xt0
                        first = False
                    else:
                        xtb = xtpool.tile([P, DT, TB], dt.float16, tag="xt")
                        eng = nc.scalar if blkctr in (2, 4) else nc.sync
                        eng.dma_start(xtb[:, :, :tb], xt_src(e, t0, tb, 0, DT))
                    t0g = seg_off[e] + t0
                    for ht in range(HT_LOC):
                        ps1 = psum.tile([P, TB], dt.float32, tag="ps")
                        for dd in range(DT):
                            nc.tensor.matmul(
                                ps1[:, :tb],
                                w1_t[:, ht, dd, :],
                                xtb[:, dd, :tb],
                                start=(dd == 0),
                                stop=(dd == DT - 1),
                            )
                        nc.scalar.activation(
                            h1_all[:, ht, t0g : t0g + tb],
                            ps1[:, :tb],
                            mybir.ActivationFunctionType.Gelu,
                            bias=b1_sb[:, e, ht : ht + 1],
                        )

            # Phase B: y partial = h1 @ W2loc.T, fp16 out (gates applied on
            # host during the combine). Output DMAs are triggered from the
            # ACT sequencer right after each block's last PSUM-copy, so the
            # kernel tail is one trigger with no cross-sequencer hop.
            for e in range(E):
                w2_t = w2pool.tile([P, DT, HT_LOC, P], dt.float16, tag="w2_t")
                nc.sync.dma_start(w2_t[:], w2s[e])
                for (t0, tb) in blocks[e]:
                    t0g = seg_off[e] + t0
                    is_last_blk = e == E - 1 and t0 == blocks[e][-1][0]
                    if is_last_blk:
                        # exact shape => the whole-tile out DMA is contiguous
                        # per partition (128 descriptors, fast trigger)
                        ot = pers.tile([P, DT, tb], dt.float16, tag="ot_last")
                    else:
                        ot = otpool.tile([P, DT, TB], dt.float16, tag="ot")
                    base = xt_off[(e, t0)]
                    dst = yt[base : base + P * DT * tb].rearrange(
                        "(p d t) -> p d t", d=DT, t=tb
                    )
                    is_last = e == E - 1 and t0 == blocks[e][-1][0]
                    for dti in range(DT):
                        ps2 = psum.tile([P, TB], dt.float32, tag="ps")
                        for ht in range(HT_LOC):
                            nc.tensor.matmul(
                                ps2[:, :tb],
                                w2_t[:, dti, ht, :],
                                h1_all[:, ht, t0g : t0g + tb],
                                start=(ht == 0),
                                stop=(ht == HT_LOC - 1),
                            )
                        if (is_last or tb <= 288) and dti % 2 == 1:
                            # narrow blocks: ACT evac alone runs at ~100% of
                            # the PE pace — split across ACT+DVE (also halves
                            # the post-last-matmul drain on the final block)
                            nc.vector.tensor_copy(ot[:, dti, :tb], ps2[:, :tb])
                        else:
                            nc.scalar.activation(
                                ot[:, dti, :tb],
                                ps2[:, :tb],
                                mybir.ActivationFunctionType.Identity,
                            )
                    if is_last:
                        # one fast contiguous trigger right after the last
                        # evac, same-engine (no cross-sequencer hop)
                        nc.scalar.dma_start(dst[:], ot[:])
                    else:
                        # SP is idle during phase B; keep ACT free for the
                        # PSUM evacuations
                        nc.sync.dma_start(dst[:], ot[:, :, :tb])

    nc.compile()
    return nc


# Results of the most recent device run (for test harness introspection).
LAST_RESULTS = None


def kernel(x, Wr, br, W1, b1, W2, b2):
    global LAST_RESULTS
    _install_axon_hooks_shim()
    from concourse.bass_utils import run_bass_kernel_spmd

    x = np.asarray(x, dtype=np.float32)
    Wr = np.asarray(Wr, dtype=np.float32)
    br = np.asarray(br, dtype=np.float32)
    W1 = np.asarray(W1, dtype=np.float32)
    b1 = np.asarray(b1, dtype=np.float32)
    W2 = np.asarray(W2, dtype=np.float32)
    b2 = np.asarray(b2, dtype=np.float32)

    B, S, Din = x.shape
    assert Din == D
    T = B * S
    x_flat = x.reshape(T, D)

    # --- Router (host, fp32, matches reference semantics) ---
    logits = x_flat @ Wr.T + br
    m = logits.max(axis=-1, keepdims=True)
    p = np.exp(logits - m)
    gates = p / p.sum(axis=-1, keepdims=True)
    # top-k, descending, ties -> lower index (matches jax.lax.top_k)
    top_i = np.argsort(-gates, axis=-1, kind="stable")[:, :TOPK]

    # --- Dispatch plan: rows sorted by expert, padded to 16 per expert ---
    sel = np.zeros((T, E), dtype=bool)
    sel[np.arange(T)[:, None], top_i] = True
    idx_list = [np.flatnonzero(sel[:, e]) for e in range(E)]
    counts = np.array([len(ix) for ix in idx_list])
    c16s = tuple(int(-(-c // 16) * 16) for c in counts)
    seg_off = np.concatenate([[0], np.cumsum(c16s)]).astype(np.int64)
    C = int(seg_off[-1])

    # global row index of (token, k) pair in the packed layout
    pos = np.empty((T, TOPK), dtype=np.int64)
    for e in range(E):
        ix = idx_list[e]
        rowmap = seg_off[e] + np.arange(len(ix))
        for k in range(TOPK):
            mask = top_i[:, k] == e
            # rows of ix that correspond to mask: ix is sorted token ids of
            # this expert; mask selects tokens routed to e at slot k
            sel_tok = np.flatnonzero(mask)
            pos[sel_tok, k] = rowmap[np.searchsorted(ix, sel_tok)]

    # --- Packed activations (identical for all cores) ---
    f16 = np.float16
    xt_parts = []
    for e in range(E):
        ix = idx_list[e]
        n = len(ix)
        xe = np.zeros((c16s[e], D), dtype=f16)
        xe[:n] = x_flat[ix]
        for (t0, tb) in _blocks_for(
            c16s[e], first_seg=(e == 0), last_seg=(e == E - 1)
        ):
            blk = xe[t0 : t0 + tb]  # [tb, D]
            xt_parts.append(
                np.ascontiguousarray(
                    blk.reshape(tb, DT, P).transpose(2, 1, 0)
                ).reshape(-1)
            )
    xt_all = np.concatenate(xt_parts)

    # --- Per-core weight slices ---
    HS = H // NCORES
    in_maps = []
    for mcore in range(NCORES):
        h0 = mcore * HS
        w1c = np.ascontiguousarray(
            W1[:, h0 : h0 + HS, :]
            .reshape(E, HT_LOC, P, DT, P)
            .transpose(0, 4, 1, 3, 2)
        ).astype(f16)
        w2c = np.ascontiguousarray(
            W2[:, :, h0 : h0 + HS]
            .reshape(E, DT, P, HT_LOC, P)
            .transpose(0, 4, 1, 3, 2)
        ).astype(f16)
        b1c = np.ascontiguousarray(
            b1[:, h0 : h0 + HS].reshape(E, HT_LOC, P).transpose(2, 0, 1)
        ).astype(np.float32)
        in_maps.append({"xt": xt_all, "w1s": w1c, "w2s": w2c, "b1t": b1c})

    # --- Compile (cached) + run on 8 cores ---
    if c16s not in _KERNEL_CACHE:
        _KERNEL_CACHE[c16s] = _build_kernel(c16s)
    nc = _KERNEL_CACHE[c16s]

    trace = bool(int(os.environ.get("MOE_KERNEL_TRACE", "0")))
    res = None
    last_exc = None
    for attempt in range(3):
        try:
            res = run_bass_kernel_spmd(
                nc, in_maps, core_ids=list(range(NCORES)), trace=trace
            )
            break
        except Exception as e:  # transient axon/NRT hiccups — retry
            last_exc = e
            trace = False
    if res is None:
        raise last_exc
    LAST_RESULTS = res

    # --- Combine (host): unpack blocks, sum partials, apply gates ---
    ysum = np.zeros((C, D), dtype=np.float32)
    for mcore in range(NCORES):
        ysum += _unpack_yt(res.results[mcore]["yt"], c16s)
    g = gates[np.arange(T)[:, None], top_i]  # [T, K]
    out_flat = (
        g[:, 0:1] * ysum[pos[:, 0]] + g[:, 1:2] * ysum[pos[:, 1]]
    ).astype(np.float32)
    if np.any(b2):
        out_flat += g[:, 0:1] * b2[top_i[:, 0]] + g[:, 1:2] * b2[top_i[:, 1]]
    return out_flat.reshape(B, S, Din)


def _unpack_yt(yt_flat, c16s):
    """Inverse of the xt block packing: flat (p, d, t) blocks -> [C, D]."""
    y = np.empty((int(sum(c16s)), D), dtype=np.float32)
    seg = 0
    off = 0
    for e in range(E):
        for (t0, tb) in _blocks_for(
            c16s[e], first_seg=(e == 0), last_seg=(e == E - 1)
        ):
            n = P * DT * tb
            chunk = yt_flat[off : off + n].reshape(P, DT, tb)
            y[seg + t0 : seg + t0 + tb] = (
                chunk.transpose(2, 1, 0).reshape(tb, D).astype(np.float32)
            )
            off += n
        seg += c16s[e]
    return y

